# revision 3
# baseline (speedup 1.0000x reference)
"""MD5Surrogate Bass kernel for 8x TRN2 NeuronCores.

Strategy: pure data-parallel over batch (2048 rows/core), each core runs the
64-round scan locally. Within a core the batch is split into 2 independent
1024-wide "streams" so the sequential round recurrence pipelines across the
Tensor (matmuls), Scalar (gelu), and Vector (state epilogue) engines.

Layout: activations are stored feature-on-partition [feat, batch]; weights
load directly as [K, M] stationary tiles. All matmuls run in float32r
(full-rate fp32, ~3e-4 relative rounding). The per-round bias b1 and the
rinfo columns of W1 are folded into an augmented 21-row L1 weight (state 16
rows + word 4 rows + const-1 row). L2's bias is applied by the Scalar
engine's per-partition bias during gelu; L3's bias on the Vector engine.
"""

import sys
import os

sys.path.insert(0, "/opt/trn_rl_repo")

import numpy as np

NUM_ROUNDS = 64
DH = 256
B = 16384
NCORES = 8
BC = B // NCORES          # batch per core = 2048
NSTREAM = 2
SW = BC // NSTREAM        # stream width = 1024

# MD5 message schedule g(i) and shift amounts (deterministic, hardcoded)
_SCHED = np.array(
    [i if i < 16 else ((5 * i + 1) % 16 if i < 32 else ((3 * i + 5) % 16 if i < 48 else (7 * i) % 16))
     for i in range(64)],
    dtype=np.int32,
)
_SHIFT = np.array(
    [7, 12, 17, 22] * 4 + [5, 9, 14, 20] * 4 + [4, 11, 16, 23] * 4 + [6, 10, 15, 21] * 4,
    dtype=np.float32,
)
_ROUND_INFO = np.stack(
    [np.arange(64, dtype=np.float32) / 64.0, _SHIFT / 25.0], axis=-1
)  # (64, 2)

# slab free-dim layout (per round, [128, SLAB_F] fp32r):
#   [0:512)    W2 chunks, chunk c=2k+j is W2[k*128:(k+1)*128, j*128:(j+1)*128]
#   [512:544)  W3 chunks, chunk k is W3[k*128:(k+1)*128, :]
#   [544:546)  b2 as (128, 2), column j = b2[j*128:(j+1)*128]
#   [546:547)  b3 in partitions 0:16
#   [547:803)  W1p (21 partitions x 256): rows = [W1[:16], b1', W1[16:20]]
SLAB_F = 804

_COMPILED = {}


def _build(time_loop_iters=0):
    import concourse.bass as bass  # noqa: F401
    from concourse import bacc
    import concourse.mybir as mybir
    from concourse.tile import TileContext

    F32 = mybir.dt.float32
    F32R = mybir.dt.float32r
    AF = mybir.ActivationFunctionType

    nc = bacc.Bacc()
    msg_d = nc.dram_tensor("msg", [64, BC], F32R, kind="ExternalInput")
    st0_d = nc.dram_tensor("st0", [17, BC], F32R, kind="ExternalInput")
    slab_d = nc.dram_tensor("slab", [NUM_ROUNDS, 128, SLAB_F], F32R, kind="ExternalInput")
    out_d = nc.dram_tensor("out", [16, BC], F32, kind="ExternalOutput")

    with TileContext(nc) as tc:
        with tc.tile_pool(name="cpool", bufs=1) as cpool, \
             tc.tile_pool(name="wpool", bufs=4) as wpool, \
             tc.tile_pool(name="pspool", bufs=1, space="PSUM") as ps:

            xS = []
            h1S = []
            h2S = []
            for s in range(NSTREAM):
                x = cpool.tile([21, SW], F32R, name=f"x{s}")
                h1 = cpool.tile([128, 2 * SW], F32R, name=f"h1{s}")
                h2 = cpool.tile([128, 2 * SW], F32R, name=f"h2{s}")
                xS.append(x)
                h1S.append(h1)
                h2S.append(h2)
                nc.sync.dma_start(x[0:17, :], st0_d[:, s * SW:(s + 1) * SW])
            outt = cpool.tile([16, BC], F32, name="outt")

            def one_round(i, is_last):
                slab = wpool.tile([128, SLAB_F], F32R, tag="slab", name="slab")
                nc.sync.dma_start(slab[:], slab_d[i])
                W2v = slab[:, 0:512]
                W3v = slab[:, 512:544]
                b2v = slab[:, 544:546].bitcast(F32)
                b3v = slab[0:16, 546:547].bitcast(F32)
                W1v = slab[0:21, 547:803]
                g = int(_SCHED[i])
                for s in range(NSTREAM):
                    x, h1, h2 = xS[s], h1S[s], h2S[s]
                    nc.sync.dma_start(
                        x[17:21, :], msg_d[4 * g:4 * g + 4, s * SW:(s + 1) * SW])
                    # L1: h1 = gelu(W1p.T @ x)   (bias folded via const row)
                    for j in range(2):
                        ps1 = ps.tile([128, SW], F32, tag="psA", bufs=3, name="ps1")
                        for b in range(2):
                            nc.tensor.matmul(
                                ps1[:, b * 512:(b + 1) * 512],
                                W1v[:, j * 128:(j + 1) * 128],
                                x[:, b * 512:(b + 1) * 512],
                                start=True, stop=True)
                        nc.scalar.activation(
                            h1[:, j * SW:(j + 1) * SW], ps1[:], AF.Gelu)
                    # L2: h2 = gelu(W2.T @ h1 + b2)
                    for j in range(2):
                        ps2 = ps.tile([128, SW], F32, tag="psA", bufs=3, name="ps2")
                        for b in range(2):
                            for k in range(2):
                                nc.tensor.matmul(
                                    ps2[:, b * 512:(b + 1) * 512],
                                    W2v[:, (2 * k + j) * 128:(2 * k + j + 1) * 128],
                                    h1[:, k * SW + b * 512:k * SW + (b + 1) * 512],
                                    start=(k == 0), stop=(k == 1))
                        nc.scalar.activation(
                            h2[:, j * SW:(j + 1) * SW], ps2[:], AF.Gelu,
                            bias=b2v[:, j:j + 1])
                    # L3: state' = W3.T @ h2 + b3  -> x rows 0:16 (or out)
                    for b in range(2):
                        ps3 = ps.tile([16, 512], F32, tag="psB", bufs=2, name="ps3")
                        for k in range(2):
                            nc.tensor.matmul(
                                ps3[:],
                                W3v[:, k * 16:(k + 1) * 16],
                                h2[:, k * SW + b * 512:k * SW + (b + 1) * 512],
                                start=(k == 0), stop=(k == 1))
                        if is_last:
                            nc.vector.tensor_scalar_add(
                                outt[:, s * SW + b * 512:s * SW + (b + 1) * 512],
                                ps3[:], b3v)
                        else:
                            nc.vector.tensor_scalar_add(
                                x[0:16, b * 512:(b + 1) * 512], ps3[:], b3v)

            if time_loop_iters:
                with tc.For_i(0, time_loop_iters, 1):
                    for i in range(NUM_ROUNDS):
                        one_round(i, i == NUM_ROUNDS - 1)
            else:
                for i in range(NUM_ROUNDS):
                    one_round(i, i == NUM_ROUNDS - 1)

            nc.sync.dma_start(out_d[:], outt[:])

    nc.compile()
    return nc


def _prep_host(message_bytes, initial_state, W1, b1, W2, b2, W3, b3):
    message_bytes = np.asarray(message_bytes, dtype=np.float32)
    initial_state = np.asarray(initial_state, dtype=np.float32)
    W1 = np.asarray(W1, dtype=np.float32)
    b1 = np.asarray(b1, dtype=np.float32)
    W2 = np.asarray(W2, dtype=np.float32)
    b2 = np.asarray(b2, dtype=np.float32)
    W3 = np.asarray(W3, dtype=np.float32)
    b3 = np.asarray(b3, dtype=np.float32)

    msg_t = np.ascontiguousarray(message_bytes.T)      # (64, B)
    st0_t = np.concatenate(
        [initial_state.T, np.ones((1, B), dtype=np.float32)], axis=0)  # (17, B)

    slab = np.zeros((NUM_ROUNDS, 128, SLAB_F), dtype=np.float32)
    for i in range(NUM_ROUNDS):
        W2i = W2[i]
        slab[i, :, 0:128] = W2i[0:128, 0:128]
        slab[i, :, 128:256] = W2i[0:128, 128:256]
        slab[i, :, 256:384] = W2i[128:256, 0:128]
        slab[i, :, 384:512] = W2i[128:256, 128:256]
        W3i = W3[i]
        slab[i, :, 512:528] = W3i[0:128, :]
        slab[i, :, 528:544] = W3i[128:256, :]
        slab[i, :, 544:546] = b2[i].reshape(2, 128).T
        slab[i, 0:16, 546] = b3[i]
        b1p = b1[i] + _ROUND_INFO[i] @ W1[i][20:22]    # fold rinfo into bias
        slab[i, 0:16, 547:803] = W1[i][0:16]
        slab[i, 16, 547:803] = b1p
        slab[i, 17:21, 547:803] = W1[i][16:20]
    return msg_t, st0_t, slab


def kernel(message_bytes, initial_state, W1, b1, W2, b2, W3, b3):
    from concourse.bass_utils import run_bass_kernel_spmd

    if "nc" not in _COMPILED:
        _COMPILED["nc"] = _build()
    nc = _COMPILED["nc"]

    msg_t, st0_t, slab = _prep_host(
        message_bytes, initial_state, W1, b1, W2, b2, W3, b3)

    in_maps = []
    for c in range(NCORES):
        sl = slice(c * BC, (c + 1) * BC)
        in_maps.append({
            "msg": np.ascontiguousarray(msg_t[:, sl]),
            "st0": np.ascontiguousarray(st0_t[:, sl]),
            "slab": slab,
        })
    res = run_bass_kernel_spmd(nc, in_maps, list(range(NCORES)))
    out = np.concatenate([res.results[c]["out"] for c in range(NCORES)], axis=1)
    return np.ascontiguousarray(out.T)  # (B, 16) float32


# revision 4
# speedup vs baseline: 2.1630x; 2.1630x over previous
"""MD5Surrogate Bass kernel for 8x TRN2 NeuronCores.

Strategy: pure data-parallel over batch (2048 rows/core), each core runs the
64-round scan locally. Within a core the batch is split into 2 independent
1024-wide "streams" so the sequential round recurrence pipelines across the
Tensor (matmuls), Scalar (gelu), and Vector (state epilogue) engines.

Layout: activations are stored feature-on-partition [feat, batch]; weights
load directly as [K, M] stationary tiles. All matmuls run in float32r
(full-rate fp32, ~3e-4 relative rounding). The per-round bias b1 and the
rinfo columns of W1 are folded into an augmented 21-row L1 weight (state 16
rows + word 4 rows + const-1 row). L2's bias is applied by the Scalar
engine's per-partition bias during gelu; L3's bias on the Vector engine.
"""

import sys
import os

sys.path.insert(0, "/opt/trn_rl_repo")

import numpy as np

NUM_ROUNDS = 64
DH = 256
B = 16384
NCORES = 8
BC = B // NCORES          # batch per core = 2048
NSTREAM = 2
SW = BC // NSTREAM        # stream width = 1024

# MD5 message schedule g(i) and shift amounts (deterministic, hardcoded)
_SCHED = np.array(
    [i if i < 16 else ((5 * i + 1) % 16 if i < 32 else ((3 * i + 5) % 16 if i < 48 else (7 * i) % 16))
     for i in range(64)],
    dtype=np.int32,
)
_SHIFT = np.array(
    [7, 12, 17, 22] * 4 + [5, 9, 14, 20] * 4 + [4, 11, 16, 23] * 4 + [6, 10, 15, 21] * 4,
    dtype=np.float32,
)
_ROUND_INFO = np.stack(
    [np.arange(64, dtype=np.float32) / 64.0, _SHIFT / 25.0], axis=-1
)  # (64, 2)

# slab free-dim layout (per round, [128, SLAB_F] fp32r):
#   [0:512)    W2 chunks, chunk c=2k+j is W2[k*128:(k+1)*128, j*128:(j+1)*128]
#   [512:544)  W3 chunks, chunk k is W3[k*128:(k+1)*128, :]
#   [544:546)  b2 as (128, 2), column j = b2[j*128:(j+1)*128]
#   [546:547)  b3 in partitions 0:16
#   [547:803)  W1p (21 partitions x 256): rows = [W1[:16], b1', W1[16:20]]
SLAB_F = 804

_COMPILED = {}


def _build(time_loop_iters=0):
    import concourse.bass as bass  # noqa: F401
    from concourse import bacc
    import concourse.mybir as mybir
    from concourse.tile import TileContext

    F32 = mybir.dt.float32
    F32R = mybir.dt.float32r
    AF = mybir.ActivationFunctionType

    nc = bacc.Bacc()
    msg_d = nc.dram_tensor("msg", [64, BC], F32R, kind="ExternalInput")
    st0_d = nc.dram_tensor("st0", [17, BC], F32R, kind="ExternalInput")
    slab_d = nc.dram_tensor("slab", [NUM_ROUNDS, 128, SLAB_F], F32R, kind="ExternalInput")
    out_d = nc.dram_tensor("out", [16, BC], F32, kind="ExternalOutput")

    with TileContext(nc) as tc:
        with tc.tile_pool(name="cpool", bufs=1) as cpool, \
             tc.tile_pool(name="wpool", bufs=6) as wpool, \
             tc.tile_pool(name="pspool", bufs=1, space="PSUM") as ps:

            xS = []
            h1S = []
            h2S = []
            for s in range(NSTREAM):
                x = cpool.tile([21, SW], F32R, name=f"x{s}")
                h1 = cpool.tile([128, 2 * SW], F32R, name=f"h1{s}")
                h2 = cpool.tile([128, 2 * SW], F32R, name=f"h2{s}")
                xS.append(x)
                h1S.append(h1)
                h2S.append(h2)
                nc.sync.dma_start(x[0:17, :], st0_d[:, s * SW:(s + 1) * SW])
            outt = cpool.tile([16, BC], F32, name="outt")

            def one_round(i, is_last):
                slab = wpool.tile([128, SLAB_F], F32R, tag="slab", name="slab")
                nc.sync.dma_start(slab[:], slab_d[i])
                W2v = slab[:, 0:512]
                W3v = slab[:, 512:544]
                b2v = slab[:, 544:546].bitcast(F32)
                b3v = slab[0:16, 546:547].bitcast(F32)
                W1v = slab[0:21, 547:803]
                g = int(_SCHED[i])
                for s in range(NSTREAM):
                    nc.sync.dma_start(
                        xS[s][17:21, :], msg_d[4 * g:4 * g + 4, s * SW:(s + 1) * SW])
                for s in range(NSTREAM):
                    x, h1, h2 = xS[s], h1S[s], h2S[s]
                    # L1: h1 = gelu(W1p.T @ x)   (bias folded via const row)
                    for j in range(2):
                        ps1 = ps.tile([128, SW], F32, tag="psA", bufs=3, name="ps1")
                        for b in range(2):
                            nc.tensor.matmul(
                                ps1[:, b * 512:(b + 1) * 512],
                                W1v[:, j * 128:(j + 1) * 128],
                                x[:, b * 512:(b + 1) * 512],
                                start=True, stop=True)
                        nc.scalar.activation(
                            h1[:, j * SW:(j + 1) * SW], ps1[:], AF.Gelu)
                    # L2: h2 = gelu(W2.T @ h1 + b2)
                    for j in range(2):
                        ps2 = ps.tile([128, SW], F32, tag="psA", bufs=3, name="ps2")
                        for b in range(2):
                            for k in range(2):
                                nc.tensor.matmul(
                                    ps2[:, b * 512:(b + 1) * 512],
                                    W2v[:, (2 * k + j) * 128:(2 * k + j + 1) * 128],
                                    h1[:, k * SW + b * 512:k * SW + (b + 1) * 512],
                                    start=(k == 0), stop=(k == 1))
                        nc.scalar.activation(
                            h2[:, j * SW:(j + 1) * SW], ps2[:], AF.Gelu,
                            bias=b2v[:, j:j + 1])
                    # L3: state' = W3.T @ h2 + b3  -> x rows 0:16 (or out)
                    for b in range(2):
                        ps3 = ps.tile([16, 512], F32, tag="psB", bufs=2, name="ps3")
                        for k in range(2):
                            nc.tensor.matmul(
                                ps3[:],
                                W3v[:, k * 16:(k + 1) * 16],
                                h2[:, k * SW + b * 512:k * SW + (b + 1) * 512],
                                start=(k == 0), stop=(k == 1))
                        if is_last:
                            nc.vector.tensor_scalar_add(
                                outt[:, s * SW + b * 512:s * SW + (b + 1) * 512],
                                ps3[:], b3v)
                        else:
                            nc.vector.tensor_scalar_add(
                                x[0:16, b * 512:(b + 1) * 512], ps3[:], b3v)

            if time_loop_iters:
                with tc.For_i(0, time_loop_iters, 1):
                    for i in range(NUM_ROUNDS):
                        one_round(i, i == NUM_ROUNDS - 1)
            else:
                for i in range(NUM_ROUNDS):
                    one_round(i, i == NUM_ROUNDS - 1)

            nc.sync.dma_start(out_d[:], outt[:])

    nc.compile()
    return nc


def _prep_host(message_bytes, initial_state, W1, b1, W2, b2, W3, b3):
    message_bytes = np.asarray(message_bytes, dtype=np.float32)
    initial_state = np.asarray(initial_state, dtype=np.float32)
    W1 = np.asarray(W1, dtype=np.float32)
    b1 = np.asarray(b1, dtype=np.float32)
    W2 = np.asarray(W2, dtype=np.float32)
    b2 = np.asarray(b2, dtype=np.float32)
    W3 = np.asarray(W3, dtype=np.float32)
    b3 = np.asarray(b3, dtype=np.float32)

    msg_t = np.ascontiguousarray(message_bytes.T)      # (64, B)
    st0_t = np.concatenate(
        [initial_state.T, np.ones((1, B), dtype=np.float32)], axis=0)  # (17, B)

    slab = np.zeros((NUM_ROUNDS, 128, SLAB_F), dtype=np.float32)
    for i in range(NUM_ROUNDS):
        W2i = W2[i]
        slab[i, :, 0:128] = W2i[0:128, 0:128]
        slab[i, :, 128:256] = W2i[0:128, 128:256]
        slab[i, :, 256:384] = W2i[128:256, 0:128]
        slab[i, :, 384:512] = W2i[128:256, 128:256]
        W3i = W3[i]
        slab[i, :, 512:528] = W3i[0:128, :]
        slab[i, :, 528:544] = W3i[128:256, :]
        slab[i, :, 544:546] = b2[i].reshape(2, 128).T
        slab[i, 0:16, 546] = b3[i]
        b1p = b1[i] + _ROUND_INFO[i] @ W1[i][20:22]    # fold rinfo into bias
        slab[i, 0:16, 547:803] = W1[i][0:16]
        slab[i, 16, 547:803] = b1p
        slab[i, 17:21, 547:803] = W1[i][16:20]
    return msg_t, st0_t, slab


def kernel(message_bytes, initial_state, W1, b1, W2, b2, W3, b3):
    from concourse.bass_utils import run_bass_kernel_spmd

    if "nc" not in _COMPILED:
        _COMPILED["nc"] = _build()
    nc = _COMPILED["nc"]

    msg_t, st0_t, slab = _prep_host(
        message_bytes, initial_state, W1, b1, W2, b2, W3, b3)

    in_maps = []
    for c in range(NCORES):
        sl = slice(c * BC, (c + 1) * BC)
        in_maps.append({
            "msg": np.ascontiguousarray(msg_t[:, sl]),
            "st0": np.ascontiguousarray(st0_t[:, sl]),
            "slab": slab,
        })
    res = run_bass_kernel_spmd(nc, in_maps, list(range(NCORES)))
    out = np.concatenate([res.results[c]["out"] for c in range(NCORES)], axis=1)
    return np.ascontiguousarray(out.T)  # (B, 16) float32
